# revision 7
# baseline (speedup 1.0000x reference)
"""C2Q attention kernel for 8 TRN2 NeuronCores.

Math (per batch):
    u      = (o_q @ W.T + b) / sqrt(H)          [Tq, H]
    score  = o_c @ u.T                           [Tc, Tq]
    prob   = softmax_j(score masked at j>=q_len) [Tc, Tq]
    out    = (prob * (i < c_len)) @ o_q          [Tc, H]

Device layout choices (everything lands K-on-partitions with zero on-chip
transposes of activations):
    u computed as [o, j]  (lhsT = W.T[h, o] tile, rhs = o_qT[h, j])
    score computed TRANSPOSED e=[j, i] (lhsT = u[o, j-block], rhs = o_cT[o, i])
    exp via ACT with per-partition bias qb[j] in {0, -6e4}: masked -> exactly 0
    denominator d[1, i] = ones[j,1].T @ e  (matmul partition-reduce)
    1/d transposed to columns via K=1 matmuls, folded into context eviction
    context [i, h] = e[j, i-block].T @ o_q[j, h]   (natural output layout)
c_len row masking is applied host-side (those rows are zeroed, never read).

Ragged specialization: the program is built AFTER the inputs are known, so
per-batch tile counts jt=ceil(q_len/128), it=ceil(c_len/128) are baked in.
Batches are assigned to (core, slot) pairs; SPMD requires every core to run
the same program, so slot s uses the max (J_s, I_s) over the 8 batches
assigned to it. The assignment is optimized to minimize total padded work.
Masked-out j tiles (j >= J*128) are never computed: their e would be exactly
zero (exp bias), so they contribute nothing to the denominator or context.
Rows i in [c_len, I*128) are computed but never read by the host gather.
"""

import os
import sys

import numpy as np

if "/opt/trn_rl_repo" not in sys.path:
    sys.path.insert(0, "/opt/trn_rl_repo")

B, Tc, Tq, H = 32, 512, 512, 1024
N_CORES = 8
N_SLOTS = B // N_CORES  # 4 batches per core, one per slot
KT = H // 128  # contraction tiles over h (8)
OT = H // 128  # linear-output tiles over o (8)
HB = H // 512  # free-dim blocks for context matmul (2)
SCALE = 1.0 / 32.0  # 1/sqrt(H)
OUT16 = bool(int(os.environ.get("K_OUT16", "1")))
KMAJOR = bool(int(os.environ.get("K_KMAJOR", "1")))
BATCHR = bool(int(os.environ.get("K_BATCHR", "1")))


def _slot_cost(j, i):
    # PE rows: Linear 64*j*128, score j*8*i*128, d i*128, ctx i*2*j*512
    return 8192 * j + 2048 * j * i + 128 * i


def _assign_slots(jt, it):
    """Partition the B batches into N_SLOTS groups of N_CORES, minimizing
    sum over groups of _slot_cost(maxJ, maxI). Returns list of groups
    (each a list of batch indices), sorted largest-cost first."""
    idx = list(range(B))
    best = None
    for key in (
        lambda b: (jt[b], it[b]),
        lambda b: (it[b], jt[b]),
        lambda b: _slot_cost(jt[b], it[b]),
    ):
        order = sorted(idx, key=key, reverse=True)
        slots = [order[s * N_CORES : (s + 1) * N_CORES] for s in range(N_SLOTS)]

        def tot(slots):
            return sum(
                _slot_cost(max(jt[b] for b in g), max(it[b] for b in g))
                for g in slots
            )

        # pairwise-swap hill climb
        improved = True
        while improved:
            improved = False
            for s1 in range(N_SLOTS):
                for s2 in range(s1 + 1, N_SLOTS):
                    for a in range(N_CORES):
                        for b2 in range(N_CORES):
                            cur = tot(slots)
                            slots[s1][a], slots[s2][b2] = (
                                slots[s2][b2],
                                slots[s1][a],
                            )
                            if tot(slots) < cur:
                                improved = True
                            else:
                                slots[s1][a], slots[s2][b2] = (
                                    slots[s2][b2],
                                    slots[s1][a],
                                )
            if best is None or tot(slots) < best[0]:
                best = (tot(slots), [list(g) for g in slots])
    slots = best[1]
    slots.sort(
        key=lambda g: _slot_cost(max(jt[b] for b in g), max(it[b] for b in g)),
        reverse=True,
    )
    return slots


def _build_program(slot_shapes):
    """slot_shapes: list of (J, I) per slot; one batch per core per slot."""
    import concourse.bacc as bacc
    import concourse.mybir as mybir
    import concourse.tile as tile

    f32 = mybir.dt.float32
    f16 = mybir.dt.float16

    nc = bacc.Bacc("TRN2", debug=False)

    WTW = H  # wt slab width
    oqT_d, ocT_d, oqN_d, out_d = [], [], [], []
    for s, (J, I) in enumerate(slot_shapes):
        qc, ic = J * 128, I * 128
        oqT_d.append(
            nc.declare_dram_parameter(f"oqT{s}", [KT, 128, qc + 8], f16, isOutput=False)
        )
        ocT_d.append(
            nc.declare_dram_parameter(f"ocT{s}", [KT, 128, ic + 8], f16, isOutput=False)
        )
        oqN_d.append(
            nc.declare_dram_parameter(f"oqN{s}", [qc, H], f16, isOutput=False)
        )
        out_d.append(
            nc.declare_dram_parameter(f"out{s}", [ic, H], f16 if OUT16 else f32, isOutput=True)
        )
    wt_d = nc.declare_dram_parameter("wt", [KT, 128, WTW], f16, isOutput=False)
    bias_d = nc.declare_dram_parameter("biasP", [128, OT], f32, isOutput=False)

    with tile.TileContext(nc) as tc:
        with (
            tc.tile_pool(name="const", bufs=1) as cpool,
            tc.tile_pool(name="inp", bufs=2) as ipool,
            tc.tile_pool(name="work", bufs=1) as wpool,
            tc.tile_pool(name="outp", bufs=3) as opool,
            tc.tile_pool(name="ps_u", bufs=2, space="PSUM") as ps_u,
            tc.tile_pool(name="ps_s", bufs=2, space="PSUM") as ps_s,
            tc.tile_pool(name="ps_c", bufs=3, space="PSUM") as ps_c,
            tc.tile_pool(name="ps_d", bufs=1, space="PSUM") as ps_d,
        ):
            ones_s = cpool.tile([1, 1], f32)
            nc.vector.memset(ones_s, 1.0)

            # W tiles: one tile per k so the first matmuls depend only on the
            # first slices; DMAs interleaved with slot-0 oqT below.
            wt_k = [cpool.tile([128, WTW], f16, tag=f"wt{k}", name=f"wt{k}") for k in range(KT)]
            biasP = cpool.tile([128, OT], f32)

            for s, (J, I) in enumerate(slot_shapes):
                qc, ic = J * 128, I * 128
                # per-k tiles keep DMA->matmul deps fine-grained during ramp
                oqT_k = [ipool.tile([128, qc + 8], f16, tag=f"oqT{k}", name=f"oqT{k}_{s}") for k in range(KT)]
                ocT_k = [ipool.tile([128, ic + 8], f16, tag=f"ocT{k}", name=f"ocT{k}_{s}") for k in range(KT)]
                oqN = ipool.tile([128, J, H], f16, tag="oqN", name=f"oqN_{s}")
                qb = oqT_k[KT - 1][:, qc : qc + J]
                ones = ocT_k[0][:, ic : ic + 1]
                if s == 0:
                    # one tiny DMA (~0.65us of descriptor stream) ahead of the
                    # bulk: the first Linear evictions depend on it
                    nc.sync.dma_start(out=biasP, in_=bias_d[:, :])
                for k in range(KT):
                    if s == 0:
                        nc.sync.dma_start(out=wt_k[k], in_=wt_d[k])
                    nc.sync.dma_start(out=oqT_k[k], in_=oqT_d[s][k])
                for k in range(KT):
                    nc.sync.dma_start(out=ocT_k[k], in_=ocT_d[s][k])
                for j in range(J):
                    nc.sync.dma_start(
                        out=oqN[:, j, :], in_=oqN_d[s][j * 128 : (j + 1) * 128, :]
                    )

                # ---- Linear: u[o, j] = W'@o_q.T + b'  (W', b' pre-scaled by
                # 1/32 on host).
                u = wpool.tile([128, OT, qc], f16, tag="u", name=f"u_{s}")
                if s == 0 and KMAJOR:
                    # k-major with all 8 PSUM banks as accumulators: the PE
                    # has 8 runnable matmuls the moment each (wt_k, oqT_k)
                    # pair lands, so it stays busy through the DMA ramp.
                    ups_o = [
                        ps_u.tile([128, qc], f32, tag="ups", name="ups_a"),
                        ps_u.tile([128, qc], f32, tag="ups", name="ups_b"),
                        ps_s.tile([128, qc], f32, tag="sps", name="ups_c"),
                        ps_s.tile([128, qc], f32, tag="sps", name="ups_d"),
                        ps_c.tile([128, qc], f32, tag="cps", name="ups_e"),
                        ps_c.tile([128, qc], f32, tag="cps", name="ups_f"),
                        ps_c.tile([128, qc], f32, tag="cps", name="ups_g"),
                        ps_d.tile([128, qc], f32, tag="dmisc", name="ups_h"),
                    ]
                    for k in range(KT):
                        for o in range(OT):
                            nc.tensor.matmul(
                                ups_o[o],
                                wt_k[k][:, o * 128 : (o + 1) * 128],
                                oqT_k[k][:, :qc],
                                start=(k == 0),
                                stop=(k == KT - 1),
                            )
                    for o in range(OT):
                        nc.vector.tensor_scalar(
                            out=u[:, o, :],
                            in0=ups_o[o],
                            scalar1=biasP[:, o : o + 1],
                            scalar2=None,
                            op0=mybir.AluOpType.add,
                        )
                else:
                    for o in range(OT):
                        ups = ps_u.tile([128, qc], f32, tag="ups")
                        for k in range(KT):
                            nc.tensor.matmul(
                                ups,
                                wt_k[k][:, o * 128 : (o + 1) * 128],
                                oqT_k[k][:, :qc],
                                start=(k == 0),
                                stop=(k == KT - 1),
                            )
                        nc.vector.tensor_scalar(
                            out=u[:, o, :],
                            in0=ups,
                            scalar1=biasP[:, o : o + 1],
                            scalar2=None,
                            op0=mybir.AluOpType.add,
                        )

                # ---- score_T + exp: e[j, i] = exp((u.T @ o_cT)/32 + qbias[j]),
                # with the denominator accumulation d[1, i] = sum_j e[j, i]
                # interleaved one step behind so its chain latency hides ----
                dps = ps_d.tile([1, ic], f32, tag="dmisc", name=f"dps_{s}")
                e_tiles = []
                for jt in range(J):
                    sps = ps_s.tile([128, ic], f32, tag="sps")
                    for o in range(OT):
                        nc.tensor.matmul(
                            sps,
                            u[:, o, jt * 128 : (jt + 1) * 128],
                            ocT_k[o][:, :ic],
                            start=(o == 0),
                            stop=(o == OT - 1),
                        )
                    e = wpool.tile([128, ic], f16, tag=f"e{jt}", name=f"e{jt}_{s}")
                    nc.scalar.activation(
                        out=e,
                        in_=sps,
                        func=mybir.ActivationFunctionType.Exp,
                        bias=qb[:, jt : jt + 1],
                        scale=SCALE,
                    )
                    e_tiles.append(e)
                    if jt >= 1:
                        nc.tensor.matmul(
                            dps,
                            ones,
                            e_tiles[jt - 1],
                            start=(jt == 1),
                            stop=False,
                            skip_group_check=True,
                        )
                nc.tensor.matmul(
                    dps,
                    ones,
                    e_tiles[J - 1],
                    start=(J == 1),
                    stop=True,
                    skip_group_check=True,
                )

                osb_tiles = {}

                def ctx_group(itb, hb, J=J, s=s, e_tiles=e_tiles, oqN=oqN, osb_tiles=osb_tiles):
                    if itb not in osb_tiles:
                        osb_tiles[itb] = opool.tile(
                            [128, H], f16 if OUT16 else f32, tag="osb", name=f"osb{itb}_{s}"
                        )
                    cps = ps_c.tile([128, 512], f32, tag="cps", name=f"cps{itb}{hb}_{s}")
                    for jt in range(J):
                        nc.tensor.matmul(
                            cps,
                            e_tiles[jt][:, itb * 128 : (itb + 1) * 128],
                            oqN[:, jt, hb * 512 : (hb + 1) * 512],
                            start=(jt == 0),
                            stop=(jt == J - 1),
                        )
                    return cps

                def ctx_evict(itb, hb, cps, r, s=s, osb_tiles=osb_tiles, out=out_d[s]):
                    osb = osb_tiles[itb]
                    nc.vector.tensor_scalar(
                        out=osb[:, hb * 512 : (hb + 1) * 512],
                        in0=cps,
                        scalar1=r,
                        scalar2=None,
                        op0=mybir.AluOpType.mult,
                    )
                    nc.sync.dma_start(
                        out=out[
                            itb * 128 : (itb + 1) * 128, hb * 512 : (hb + 1) * 512
                        ],
                        in_=osb[:, hb * 512 : (hb + 1) * 512],
                    )

                # first ctx group runs while the d copy drains on DVE
                cps00 = ctx_group(0, 0)
                dsb = wpool.tile([1, ic], f32, tag="dsb", name=f"dsb_{s}")
                nc.vector.tensor_copy(out=dsb, in_=dps)

                # transpose 1/d to per-partition columns via K=1 matmuls
                # (all I columns land in one PSUM tile; one reciprocal)
                if BATCHR:
                    dcps = ps_d.tile([128, I], f32, tag="dmisc", name=f"dcps_{s}")
                    for itb in range(I):
                        nc.tensor.matmul(
                            dcps[:, itb : itb + 1],
                            dsb[:, itb * 128 : (itb + 1) * 128],
                            ones_s[0:1, 0:1],
                            start=True,
                            stop=True,
                            skip_group_check=True,
                        )
                    r_all = wpool.tile([128, I], f32, tag="r", name=f"r_{s}")
                    nc.vector.reciprocal(out=r_all, in_=dcps)
                    r_cols = [r_all[:, itb : itb + 1] for itb in range(I)]
                else:
                    r_cols = []
                    for itb in range(I):
                        dcps = ps_d.tile([128, 1], f32, tag="dmisc", name=f"dcps{itb}_{s}")
                        nc.tensor.matmul(
                            dcps,
                            dsb[:, itb * 128 : (itb + 1) * 128],
                            ones_s[0:1, 0:1],
                            start=True,
                            stop=True,
                        )
                        r = wpool.tile([128, 1], f32, tag=f"r{itb}", name=f"r{itb}_{s}")
                        nc.vector.reciprocal(out=r, in_=dcps)
                        r_cols.append(r)

                cps01 = ctx_group(0, 1)
                ctx_evict(0, 0, cps00, r_cols[0])
                ctx_evict(0, 1, cps01, r_cols[0])
                for itb in range(1, I):
                    for hb in range(HB):
                        cps = ctx_group(itb, hb)
                        ctx_evict(itb, hb, cps, r_cols[itb])

    nc.compile()
    return nc


def _host_inputs(o_c, o_q, W, b, q_lengths, slots, jt, it):
    """Build the per-core input maps (host-side sharding + re-layout).

    Linear operands (W, o_qT) ship as fp16 (same PE rate, half the
    ramp-critical DMA bytes); the 1/sqrt(H) scale is applied later as the
    Exp activation's scale argument, so W keeps its natural fp16 range.
    """
    NEG16 = np.float16(-60000.0)  # exp(x - 60000) == 0 exactly in fp32
    wt_host = np.ascontiguousarray(W.T.reshape(KT, 128, H).astype(np.float16))
    bias_host = np.ascontiguousarray(b.reshape(OT, 128).T)  # [128, o_tile] f32
    in_maps = [dict() for _ in range(N_CORES)]
    for c in range(N_CORES):
        in_maps[c]["wt"] = wt_host
        in_maps[c]["biasP"] = bias_host
    for s, grp in enumerate(slots):
        J = max(jt[g] for g in grp)
        I = max(it[g] for g in grp)
        qc, ic = J * 128, I * 128
        jidx = np.arange(J)[None, :] * 128 + np.arange(128)[:, None]  # [128, J]
        for c, g in enumerate(grp):
            oqT = np.zeros((KT, 128, qc + 8), np.float16)
            oqT[:, :, :qc] = (
                o_q[g, :qc].T.reshape(KT, 128, qc).astype(np.float16)
            )
            ql = int(q_lengths[g])
            # qb (exp bias: 0 valid / -60000 masked) rides in the last slab
            oqT[KT - 1, :, qc : qc + J] = np.where(
                jidx < ql, np.float16(0.0), NEG16
            )
            ocT = np.zeros((KT, 128, ic + 8), np.float16)
            ocT[:, :, :ic] = (
                o_c[g, :ic].T.reshape(KT, 128, ic).astype(np.float16)
            )
            ocT[0, :, ic] = 1.0  # ones column for the denominator matmul
            in_maps[c][f"oqT{s}"] = oqT
            in_maps[c][f"ocT{s}"] = ocT
            in_maps[c][f"oqN{s}"] = np.ascontiguousarray(
                o_q[g, :qc].astype(np.float16)
            )
    return in_maps


def kernel(**inputs) -> np.ndarray:
    o_c = np.asarray(inputs["o_c"], dtype=np.float32)
    o_q = np.asarray(inputs["o_q"], dtype=np.float32)
    W = np.asarray(inputs["W"], dtype=np.float32)
    b = np.asarray(inputs["b"], dtype=np.float32)
    q_lengths = np.asarray(inputs["q_lengths"]).astype(np.int64)
    c_lengths = np.asarray(inputs["c_lengths"]).astype(np.int64)

    from concourse.bass_utils import run_bass_kernel_spmd

    jt = [min(max(-(-int(q) // 128), 1), Tq // 128) for q in q_lengths]
    it = [min(max(-(-int(cl) // 128), 1), Tc // 128) for cl in c_lengths]
    if bool(int(os.environ.get("K_FULL", "0"))):
        jt = [Tq // 128] * B
        it = [Tc // 128] * B
    if bool(int(os.environ.get("K_FULLJ", "0"))):
        jt = [Tq // 128] * B
    if bool(int(os.environ.get("K_FULLI", "0"))):
        it = [Tc // 128] * B
    slots = _assign_slots(jt, it)
    slot_shapes = [
        (max(jt[g] for g in grp), max(it[g] for g in grp)) for grp in slots
    ]

    in_maps = _host_inputs(o_c, o_q, W, b, q_lengths, slots, jt, it)
    nc = _build_program(slot_shapes)

    trace = bool(int(os.environ.get("KERNEL_TRACE", "0")))
    res = run_bass_kernel_spmd(
        nc, in_maps, core_ids=list(range(N_CORES)), trace=trace
    )
    if trace:
        kernel.last_results = res

    out = np.zeros((B, Tc, H), dtype=np.float32)
    for s, grp in enumerate(slots):
        for c, g in enumerate(grp):
            dev = res.results[c][f"out{s}"]
            cl = int(c_lengths[g])
            out[g, :cl] = dev[:cl].astype(np.float32)
    return out


# revision 8
# speedup vs baseline: 1.2202x; 1.2202x over previous
"""C2Q attention kernel for 8 TRN2 NeuronCores.

Math (per batch):
    u      = (o_q @ W.T + b) / sqrt(H)          [Tq, H]
    score  = o_c @ u.T                           [Tc, Tq]
    prob   = softmax_j(score masked at j>=q_len) [Tc, Tq]
    out    = (prob * (i < c_len)) @ o_q          [Tc, H]

Device layout choices (everything lands K-on-partitions with zero on-chip
transposes of activations):
    u computed as [o, j]  (lhsT = W.T[h, o] tile, rhs = o_qT[h, j])
    score computed TRANSPOSED e=[j, i] (lhsT = u[o, j-block], rhs = o_cT[o, i])
    exp via ACT with per-partition bias qb[j] in {0, -6e4}: masked -> exactly 0
    denominator d[1, i] = ones[j,1].T @ e  (matmul partition-reduce)
    1/d transposed to columns via K=1 matmuls, folded into context eviction
    context [i, h] = e[j, i-block].T @ o_q[j, h]   (natural output layout)
c_len row masking is applied host-side (those rows are zeroed, never read).

Ragged specialization: the program is built AFTER the inputs are known, so
per-batch tile counts jt=ceil(q_len/128), it=ceil(c_len/128) are baked in.
Batches are assigned to (core, slot) pairs; SPMD requires every core to run
the same program, so slot s uses the max (J_s, I_s) over the 8 batches
assigned to it. The assignment is optimized to minimize total padded work.
Masked-out j tiles (j >= J*128) are never computed: their e would be exactly
zero (exp bias), so they contribute nothing to the denominator or context.
Rows i in [c_len, I*128) are computed but never read by the host gather.
"""

import os
import sys

import numpy as np

if "/opt/trn_rl_repo" not in sys.path:
    sys.path.insert(0, "/opt/trn_rl_repo")

B, Tc, Tq, H = 32, 512, 512, 1024
N_CORES = 8
N_SLOTS = B // N_CORES  # 4 batches per core, one per slot
KT = H // 128  # contraction tiles over h (8)
OT = H // 128  # linear-output tiles over o (8)
HB = H // 512  # free-dim blocks for context matmul (2)
SCALE = 1.0 / 32.0  # 1/sqrt(H)
OUT16 = bool(int(os.environ.get("K_OUT16", "1")))
KMAJOR = bool(int(os.environ.get("K_KMAJOR", "1")))
BATCHR = bool(int(os.environ.get("K_BATCHR", "1")))


def _slot_cost(j, i):
    # PE rows: Linear 64*j*128, score j*8*i*128, d i*128, ctx i*2*j*512
    return 8192 * j + 2048 * j * i + 128 * i


def _assign_slots(jt, it):
    """Partition the B batches into N_SLOTS groups of N_CORES, minimizing
    sum over groups of _slot_cost(maxJ, maxI). Returns list of groups
    (each a list of batch indices), sorted largest-cost first."""
    idx = list(range(B))
    best = None
    for key in (
        lambda b: (jt[b], it[b]),
        lambda b: (it[b], jt[b]),
        lambda b: _slot_cost(jt[b], it[b]),
    ):
        order = sorted(idx, key=key, reverse=True)
        slots = [order[s * N_CORES : (s + 1) * N_CORES] for s in range(N_SLOTS)]

        def tot(slots):
            return sum(
                _slot_cost(max(jt[b] for b in g), max(it[b] for b in g))
                for g in slots
            )

        # pairwise-swap hill climb
        improved = True
        while improved:
            improved = False
            for s1 in range(N_SLOTS):
                for s2 in range(s1 + 1, N_SLOTS):
                    for a in range(N_CORES):
                        for b2 in range(N_CORES):
                            cur = tot(slots)
                            slots[s1][a], slots[s2][b2] = (
                                slots[s2][b2],
                                slots[s1][a],
                            )
                            if tot(slots) < cur:
                                improved = True
                            else:
                                slots[s1][a], slots[s2][b2] = (
                                    slots[s2][b2],
                                    slots[s1][a],
                                )
            if best is None or tot(slots) < best[0]:
                best = (tot(slots), [list(g) for g in slots])
    slots = best[1]
    slots.sort(
        key=lambda g: _slot_cost(max(jt[b] for b in g), max(it[b] for b in g)),
        reverse=True,
    )
    return slots


def _build_program(slot_shapes):
    """slot_shapes: list of (J, I) per slot; one batch per core per slot."""
    import concourse.bacc as bacc
    import concourse.mybir as mybir
    import concourse.tile as tile

    f32 = mybir.dt.float32
    f16 = mybir.dt.float16

    nc = bacc.Bacc("TRN2", debug=False)

    WTW = H  # wt slab width
    oqT_d, ocT_d, oqN_d, out_d = [], [], [], []
    for s, (J, I) in enumerate(slot_shapes):
        qc, ic = J * 128, I * 128
        oqT_d.append(
            nc.declare_dram_parameter(f"oqT{s}", [KT, 128, qc + 8], f16, isOutput=False)
        )
        ocT_d.append(
            nc.declare_dram_parameter(f"ocT{s}", [KT, 128, ic + 8], f16, isOutput=False)
        )
        oqN_d.append(
            nc.declare_dram_parameter(f"oqN{s}", [qc, H], f16, isOutput=False)
        )
        out_d.append(
            nc.declare_dram_parameter(f"out{s}", [ic, H], f16 if OUT16 else f32, isOutput=True)
        )
    wt_d = nc.declare_dram_parameter("wt", [KT, 128, WTW], f16, isOutput=False)
    bias_d = nc.declare_dram_parameter("biasP", [128, OT], f32, isOutput=False)

    with tile.TileContext(nc) as tc:
        with (
            tc.tile_pool(name="const", bufs=1) as cpool,
            tc.tile_pool(name="inp", bufs=2) as ipool,
            tc.tile_pool(name="work", bufs=1) as wpool,
            tc.tile_pool(name="outp", bufs=3) as opool,
            tc.tile_pool(name="ps_u", bufs=2, space="PSUM") as ps_u,
            tc.tile_pool(name="ps_s", bufs=2, space="PSUM") as ps_s,
            tc.tile_pool(name="ps_c", bufs=3, space="PSUM") as ps_c,
            tc.tile_pool(name="ps_d", bufs=1, space="PSUM") as ps_d,
        ):
            ones_s = cpool.tile([1, 1], f32)
            nc.vector.memset(ones_s, 1.0)

            # W tiles: one tile per k so the first matmuls depend only on the
            # first slices; DMAs interleaved with slot-0 oqT below.
            wt_k = [cpool.tile([128, WTW], f16, tag=f"wt{k}", name=f"wt{k}") for k in range(KT)]
            biasP = cpool.tile([128, OT], f32)

            for s, (J, I) in enumerate(slot_shapes):
                qc, ic = J * 128, I * 128
                # per-k tiles keep DMA->matmul deps fine-grained during ramp
                oqT_k = [ipool.tile([128, qc + 8], f16, tag=f"oqT{k}", name=f"oqT{k}_{s}") for k in range(KT)]
                ocT_k = [ipool.tile([128, ic + 8], f16, tag=f"ocT{k}", name=f"ocT{k}_{s}") for k in range(KT)]
                oqN = ipool.tile([128, J, H], f16, tag="oqN", name=f"oqN_{s}")
                qb = oqT_k[KT - 1][:, qc : qc + J]
                ones = ocT_k[0][:, ic : ic + 1]
                if s == 0:
                    # one tiny DMA (~0.65us of descriptor stream) ahead of the
                    # bulk: the first Linear evictions depend on it
                    nc.sync.dma_start(out=biasP, in_=bias_d[:, :])
                for k in range(KT):
                    if s == 0:
                        nc.sync.dma_start(out=wt_k[k], in_=wt_d[k])
                    nc.sync.dma_start(out=oqT_k[k], in_=oqT_d[s][k])
                for k in range(KT):
                    nc.sync.dma_start(out=ocT_k[k], in_=ocT_d[s][k])
                for j in range(J):
                    nc.sync.dma_start(
                        out=oqN[:, j, :], in_=oqN_d[s][j * 128 : (j + 1) * 128, :]
                    )

                # ---- Linear: u[o, j] = W'@o_q.T + b'  (W', b' pre-scaled by
                # 1/32 on host).
                u = wpool.tile([128, OT, qc], f16, tag="u", name=f"u_{s}")
                if s == 0 and KMAJOR:
                    # k-major with all 8 PSUM banks as accumulators: the PE
                    # has 8 runnable matmuls the moment each (wt_k, oqT_k)
                    # pair lands, so it stays busy through the DMA ramp.
                    ups_o = [
                        ps_u.tile([128, qc], f32, tag="ups", name="ups_a"),
                        ps_u.tile([128, qc], f32, tag="ups", name="ups_b"),
                        ps_s.tile([128, qc], f32, tag="sps", name="ups_c"),
                        ps_s.tile([128, qc], f32, tag="sps", name="ups_d"),
                        ps_c.tile([128, qc], f32, tag="cps", name="ups_e"),
                        ps_c.tile([128, qc], f32, tag="cps", name="ups_f"),
                        ps_c.tile([128, qc], f32, tag="cps", name="ups_g"),
                        ps_d.tile([128, qc], f32, tag="dmisc", name="ups_h"),
                    ]
                    for k in range(KT):
                        for o in range(OT):
                            nc.tensor.matmul(
                                ups_o[o],
                                wt_k[k][:, o * 128 : (o + 1) * 128],
                                oqT_k[k][:, :qc],
                                start=(k == 0),
                                stop=(k == KT - 1),
                            )
                    for o in range(OT):
                        nc.vector.tensor_scalar(
                            out=u[:, o, :],
                            in0=ups_o[o],
                            scalar1=biasP[:, o : o + 1],
                            scalar2=None,
                            op0=mybir.AluOpType.add,
                        )
                else:
                    for o in range(OT):
                        ups = ps_u.tile([128, qc], f32, tag="ups")
                        for k in range(KT):
                            nc.tensor.matmul(
                                ups,
                                wt_k[k][:, o * 128 : (o + 1) * 128],
                                oqT_k[k][:, :qc],
                                start=(k == 0),
                                stop=(k == KT - 1),
                            )
                        nc.vector.tensor_scalar(
                            out=u[:, o, :],
                            in0=ups,
                            scalar1=biasP[:, o : o + 1],
                            scalar2=None,
                            op0=mybir.AluOpType.add,
                        )

                # ---- score_T + exp: e[j, i] = exp((u.T @ o_cT)/32 + qbias[j]),
                # with the denominator accumulation d[1, i] = sum_j e[j, i]
                # interleaved one step behind so its chain latency hides ----
                dps = ps_d.tile([1, ic], f32, tag="dmisc", name=f"dps_{s}")
                e_tiles = []
                for jt in range(J):
                    sps = ps_s.tile([128, ic], f32, tag="sps")
                    for o in range(OT):
                        nc.tensor.matmul(
                            sps,
                            u[:, o, jt * 128 : (jt + 1) * 128],
                            ocT_k[o][:, :ic],
                            start=(o == 0),
                            stop=(o == OT - 1),
                        )
                    e = wpool.tile([128, ic], f16, tag=f"e{jt}", name=f"e{jt}_{s}")
                    nc.scalar.activation(
                        out=e,
                        in_=sps,
                        func=mybir.ActivationFunctionType.Exp,
                        bias=qb[:, jt : jt + 1],
                        scale=SCALE,
                    )
                    e_tiles.append(e)
                    if jt >= 1:
                        nc.tensor.matmul(
                            dps,
                            ones,
                            e_tiles[jt - 1],
                            start=(jt == 1),
                            stop=False,
                            skip_group_check=True,
                        )
                nc.tensor.matmul(
                    dps,
                    ones,
                    e_tiles[J - 1],
                    start=(J == 1),
                    stop=True,
                    skip_group_check=True,
                )

                osb_tiles = {}

                def ctx_group(itb, hb, J=J, s=s, e_tiles=e_tiles, oqN=oqN, osb_tiles=osb_tiles):
                    if itb not in osb_tiles:
                        osb_tiles[itb] = opool.tile(
                            [128, H], f16 if OUT16 else f32, tag="osb", name=f"osb{itb}_{s}"
                        )
                    cps = ps_c.tile([128, 512], f32, tag="cps", name=f"cps{itb}{hb}_{s}")
                    for jt in range(J):
                        nc.tensor.matmul(
                            cps,
                            e_tiles[jt][:, itb * 128 : (itb + 1) * 128],
                            oqN[:, jt, hb * 512 : (hb + 1) * 512],
                            start=(jt == 0),
                            stop=(jt == J - 1),
                        )
                    return cps

                def ctx_evict(itb, hb, cps, r, s=s, osb_tiles=osb_tiles, out=out_d[s]):
                    osb = osb_tiles[itb]
                    nc.vector.tensor_scalar(
                        out=osb[:, hb * 512 : (hb + 1) * 512],
                        in0=cps,
                        scalar1=r,
                        scalar2=None,
                        op0=mybir.AluOpType.mult,
                    )
                    nc.sync.dma_start(
                        out=out[
                            itb * 128 : (itb + 1) * 128, hb * 512 : (hb + 1) * 512
                        ],
                        in_=osb[:, hb * 512 : (hb + 1) * 512],
                    )

                # first ctx group runs while the d copy drains on DVE
                cps00 = ctx_group(0, 0)
                dsb = wpool.tile([1, ic], f32, tag="dsb", name=f"dsb_{s}")
                nc.vector.tensor_copy(out=dsb, in_=dps)

                # transpose 1/d to per-partition columns via K=1 matmuls
                # (all I columns land in one PSUM tile; one reciprocal)
                if BATCHR:
                    dcps = ps_d.tile([128, I], f32, tag="dmisc", name=f"dcps_{s}")
                    for itb in range(I):
                        nc.tensor.matmul(
                            dcps[:, itb : itb + 1],
                            dsb[:, itb * 128 : (itb + 1) * 128],
                            ones_s[0:1, 0:1],
                            start=True,
                            stop=True,
                            skip_group_check=True,
                        )
                    r_all = wpool.tile([128, I], f32, tag="r", name=f"r_{s}")
                    nc.vector.reciprocal(out=r_all, in_=dcps)
                    r_cols = [r_all[:, itb : itb + 1] for itb in range(I)]
                else:
                    r_cols = []
                    for itb in range(I):
                        dcps = ps_d.tile([128, 1], f32, tag="dmisc", name=f"dcps{itb}_{s}")
                        nc.tensor.matmul(
                            dcps,
                            dsb[:, itb * 128 : (itb + 1) * 128],
                            ones_s[0:1, 0:1],
                            start=True,
                            stop=True,
                        )
                        r = wpool.tile([128, 1], f32, tag=f"r{itb}", name=f"r{itb}_{s}")
                        nc.vector.reciprocal(out=r, in_=dcps)
                        r_cols.append(r)

                cps01 = ctx_group(0, 1)
                ctx_evict(0, 0, cps00, r_cols[0])
                ctx_evict(0, 1, cps01, r_cols[0])
                for itb in range(1, I):
                    for hb in range(HB):
                        cps = ctx_group(itb, hb)
                        ctx_evict(itb, hb, cps, r_cols[itb])

    nc.compile()
    return nc


def _host_inputs(o_c, o_q, W, b, q_lengths, slots, jt, it):
    """Build the per-core input maps (host-side sharding + re-layout).

    Linear operands (W, o_qT) ship as fp16 (same PE rate, half the
    ramp-critical DMA bytes); the 1/sqrt(H) scale is applied later as the
    Exp activation's scale argument, so W keeps its natural fp16 range.
    """
    NEG16 = np.float16(-60000.0)  # exp(x - 60000) == 0 exactly in fp32
    wt_host = np.ascontiguousarray(W.T.reshape(KT, 128, H).astype(np.float16))
    bias_host = np.ascontiguousarray(b.reshape(OT, 128).T)  # [128, o_tile] f32
    in_maps = [dict() for _ in range(N_CORES)]
    for c in range(N_CORES):
        in_maps[c]["wt"] = wt_host
        in_maps[c]["biasP"] = bias_host
    for s, grp in enumerate(slots):
        J = max(jt[g] for g in grp)
        I = max(it[g] for g in grp)
        qc, ic = J * 128, I * 128
        jidx = np.arange(J)[None, :] * 128 + np.arange(128)[:, None]  # [128, J]
        for c, g in enumerate(grp):
            oqT = np.zeros((KT, 128, qc + 8), np.float16)
            oqT[:, :, :qc] = (
                o_q[g, :qc].T.reshape(KT, 128, qc).astype(np.float16)
            )
            ql = int(q_lengths[g])
            # qb (exp bias: 0 valid / -60000 masked) rides in the last slab
            oqT[KT - 1, :, qc : qc + J] = np.where(
                jidx < ql, np.float16(0.0), NEG16
            )
            ocT = np.zeros((KT, 128, ic + 8), np.float16)
            ocT[:, :, :ic] = (
                o_c[g, :ic].T.reshape(KT, 128, ic).astype(np.float16)
            )
            ocT[0, :, ic] = 1.0  # ones column for the denominator matmul
            in_maps[c][f"oqT{s}"] = oqT
            in_maps[c][f"ocT{s}"] = ocT
            in_maps[c][f"oqN{s}"] = np.ascontiguousarray(
                o_q[g, :qc].astype(np.float16)
            )
    return in_maps


def kernel(**inputs) -> np.ndarray:
    o_c = np.asarray(inputs["o_c"], dtype=np.float32)
    o_q = np.asarray(inputs["o_q"], dtype=np.float32)
    W = np.asarray(inputs["W"], dtype=np.float32)
    b = np.asarray(inputs["b"], dtype=np.float32)
    q_lengths = np.asarray(inputs["q_lengths"]).astype(np.int64)
    c_lengths = np.asarray(inputs["c_lengths"]).astype(np.int64)

    from concourse.bass_utils import run_bass_kernel_spmd

    jt = [min(max(-(-int(q) // 128), 1), Tq // 128) for q in q_lengths]
    it = [min(max(-(-int(cl) // 128), 1), Tc // 128) for cl in c_lengths]
    if bool(int(os.environ.get("K_FULL", "0"))):
        jt = [Tq // 128] * B
        it = [Tc // 128] * B
    if bool(int(os.environ.get("K_FULLJ", "0"))):
        jt = [Tq // 128] * B
    if bool(int(os.environ.get("K_FULLI", "0"))):
        it = [Tc // 128] * B
    if bool(int(os.environ.get("K_UNI3", "0"))):
        jt = [3] * B
        it = [4] * B
    if bool(int(os.environ.get("K_UNI33", "0"))):
        jt = [3] * B
        it = [3] * B
    slots = _assign_slots(jt, it)
    slot_shapes = [
        (max(jt[g] for g in grp), max(it[g] for g in grp)) for grp in slots
    ]

    in_maps = _host_inputs(o_c, o_q, W, b, q_lengths, slots, jt, it)
    nc = _build_program(slot_shapes)

    trace = bool(int(os.environ.get("KERNEL_TRACE", "0")))
    res = run_bass_kernel_spmd(
        nc, in_maps, core_ids=list(range(N_CORES)), trace=trace
    )
    if trace:
        kernel.last_results = res

    out = np.zeros((B, Tc, H), dtype=np.float32)
    for s, grp in enumerate(slots):
        for c, g in enumerate(grp):
            dev = res.results[c][f"out{s}"]
            cl = int(c_lengths[g])
            out[g, :cl] = dev[:cl].astype(np.float32)
    return out
